# revision 8
# baseline (speedup 1.0000x reference)
"""BottleneckLSTMCell fused kernel for 8 Trainium2 NeuronCores.

Sharding: data-parallel over batch (B=8 -> 1 image per core). Each core runs
the full cell for its image.

v2.1: the tensor engine keeps the dense matmuls (Wy: 28/slab, gates: 64/slab,
bf16 at 1 cyc/row) plus three of the seven depthwise-3x3 chunks as diag
matmuls. The other four depthwise chunks run off-PE as per-channel products
(ACT activation-with-scale, or DVE tensor_scalar in its 4x bf16 mode) summed
by add chains on the gpsimd/vector engines (gpsimd only supports
TensorTensor arithmetic, not scalar_tensor_tensor). Work is software-
pipelined in rounds: round r runs phase A of slab r, the x-conv of slab r+1,
and phase B of slab r-1. The one DVE dwi chain is split 6 taps (dy<=0, same
round) + 3 taps (dy=+1, next round) since its last tap row comes from the
next A-slab; it finishes early in the round so the PE gate matmuls (k=0
contraction chunk) are never blocked.
"""

import sys

if '/opt/trn_rl_repo' not in sys.path:
    sys.path.insert(0, '/opt/trn_rl_repo')

import numpy as np
import ml_dtypes

import concourse.bass as bass  # noqa: F401
from concourse import bacc
import concourse.mybir as mybir
from concourse.tile import TileContext
from concourse.bass_utils import run_bass_kernel_spmd

F32 = mybir.dt.float32
BF = mybir.dt.bfloat16
AF = mybir.ActivationFunctionType
ALU = mybir.AluOpType
BF_NP = ml_dtypes.bfloat16

B, CIN, CH, HW = 8, 320, 512, 64
PIX = HW * HW          # 4096
NCORES = 8
NCHUNK = 8             # spatial slabs of 8 rows (512 px)
XCH = [128, 128, 64]   # x channel chunk sizes (320)

TAPS = [(t // 3 - 1, t % 3 - 1) for t in range(9)]


def build_nc():
    nc = bacc.Bacc(None, target_bir_lowering=False, num_devices=NCORES)

    xd = nc.dram_tensor("x", (CIN, 66, 66), BF, kind="ExternalInput")
    hd = nc.dram_tensor("h", (CH, PIX), BF, kind="ExternalInput")
    cd = nc.dram_tensor("c", (CH, PIX), BF, kind="ExternalInput")
    wyd = nc.dram_tensor("wy", (128, 7, 512), BF, kind="ExternalInput")
    wybd = nc.dram_tensor("wyb", (128, 4), F32, kind="ExternalInput")
    wgd = nc.dram_tensor("wg", (128, 16, 512), BF, kind="ExternalInput")
    dwxwd = nc.dram_tensor("dwxw", (128, 3, 9), F32, kind="ExternalInput")
    dwiwd = nc.dram_tensor("dwiw", (128, 4, 9), F32, kind="ExternalInput")
    dwidd = nc.dram_tensor("dwid", (128, 3, 1152), BF, kind="ExternalInput")
    ccd = nc.dram_tensor("occ", (CH, PIX), F32, kind="ExternalOutput")
    chd = nc.dram_tensor("och", (CH, PIX), F32, kind="ExternalOutput")

    x_ap, h_ap, c_ap = xd.ap(), hd.ap(), cd.ap()
    cc_ap, ch_ap = ccd.ap(), chd.ap()

    with TileContext(nc) as tc:
        with tc.tile_pool(name="persist", bufs=1) as pp, \
             tc.tile_pool(name="sA", bufs=2) as sA, \
             tc.tile_pool(name="sB", bufs=2) as sB, \
             tc.tile_pool(name="psi", bufs=2, space="PSUM") as psi_p, \
             tc.tile_pool(name="psg", bufs=4, space="PSUM") as psg_p, \
             tc.tile_pool(name="psb", bufs=2, space="PSUM") as psb_p:

            # ------------- persistent tiles -------------
            i_pad = [pp.tile([128, 66, 66], BF, tag=f"ipad{m}", name=f"ipad{m}")
                     for m in range(4)]
            wy_t = pp.tile([128, 7, 512], BF, tag="wy", name="wy")
            wyb_t = pp.tile([128, 4], F32, tag="wyb", name="wyb")
            wg_t = pp.tile([128, 16, 512], BF, tag="wg", name="wg")
            dwxw_t = pp.tile([128, 3, 9], F32, tag="dwxw", name="dwxw")
            dwiw_t = pp.tile([128, 4, 9], F32, tag="dwiw", name="dwiw")
            dwid_t = pp.tile([128, 3, 1152], BF, tag="dwid", name="dwid")

            # ------------- helpers -------------
            def dma_h(n):
                ts = []
                for k in range(4):
                    t = sA.tile([128, 512], BF, tag=f"h{k}", name=f"h{k}")
                    nc.sync.dma_start(
                        out=t[:],
                        in_=h_ap[128 * k:128 * (k + 1), 512 * n:512 * (n + 1)])
                    ts.append(t)
                return ts

            def dma_xpad(n):
                ts = []
                for ci in range(3):
                    pc = XCH[ci]
                    xp = sA.tile([128, 10, 66], BF, tag=f"xp{ci}",
                                 name=f"xp{ci}")
                    nc.sync.dma_start(
                        out=xp[:pc, :, :],
                        in_=x_ap[128 * ci:128 * ci + pc, 8 * n:8 * n + 10, :])
                    ts.append(xp)
                return ts

            def xwin(xp, pc, t):
                dy, dx = TAPS[t]
                return xp[:pc, 1 + dy:9 + dy, 1 + dx:65 + dx]

            def iwin(ci, n, t, pc=128):
                dy, dx = TAPS[t]
                r0 = 8 * n
                return i_pad[ci][:pc, 1 + r0 + dy:9 + r0 + dy, 1 + dx:65 + dx]

            def stt_seg(accs, out_bf, win, w_ap, t0, t1):
                """Taps t0..t1-1 of a 9-tap chain on the vector engine.

                accs = (acc_a, acc_b) fp32 ping-pong tiles; tap 8 writes
                out_bf (bf16)."""
                for t in range(t0, t1):
                    src = accs[(t + 1) % 2][:, :, :]
                    dst = accs[t % 2][:, :, :]
                    if t == 0:
                        nc.vector.tensor_scalar_mul(dst, win(t), w_ap(t))
                    elif t == 8:
                        nc.vector.scalar_tensor_tensor(
                            out=out_bf, in0=win(t), scalar=w_ap(t), in1=src,
                            op0=ALU.mult, op1=ALU.add)
                    else:
                        nc.vector.scalar_tensor_tensor(
                            out=dst, in0=win(t), scalar=w_ap(t), in1=src,
                            op0=ALU.mult, op1=ALU.add)

            def prod_tree(on_act, add_eng, tag, win, w_ap, out_bf, pc=128):
                """Depthwise chunk as 9 per-channel products + an 8-add chain.

                Products on ACT (activation Copy with per-partition scale) or
                DVE (tensor_scalar 4x bf16 mode); adds on `add_eng`."""
                prod = sA.tile([128, 9, 512], BF, tag=f"pr{tag}",
                               name=f"pr{tag}", bufs=1)
                for t in range(9):
                    if on_act:
                        nc.scalar.activation(
                            prod[:pc, t, :], win(t), AF.Copy, scale=w_ap(t))
                    else:
                        nc.vector.tensor_scalar_mul(
                            prod[:pc, t, :], win(t), w_ap(t))
                tmps = [sA.tile([128, 512], BF, tag=f"pt{tag}{j}",
                                name=f"pt{tag}{j}") for j in range(2)]
                add_eng.tensor_add(tmps[0][:pc, :], prod[:pc, 0, :],
                                   prod[:pc, 1, :])
                for j in range(2, 8):
                    add_eng.tensor_add(tmps[(j + 1) % 2][:pc, :],
                                       tmps[j % 2][:pc, :],
                                       prod[:pc, j, :])
                add_eng.tensor_add(out_bf, tmps[0][:pc, :], prod[:pc, 8, :])

            def new_accs(tag):
                a = sA.tile([128, 8, 64], F32, tag=f"{tag}a", name=f"{tag}a")
                b = sA.tile([128, 8, 64], F32, tag=f"{tag}b", name=f"{tag}b")
                return (a, b)

            def emit_dwx(n, xps):
                """All 27 x-conv taps for slab n -> xw tiles (bf16).

                ci0: ACT products + Pool adds; ci1: DVE products + DVE adds;
                ci2 (64ch): DVE products + Pool adds."""
                xw = [sA.tile([128, 8, 64], BF, tag=f"xw{ci}", name=f"xw{ci}")
                      for ci in range(3)]
                prod_tree(True, nc.gpsimd, "x0",
                          lambda t: xwin(xps[0], 128, t),
                          lambda t: dwxw_t[:, 0, t:t + 1], xw[0][:, :, :])
                prod_tree(False, nc.vector, "x1",
                          lambda t: xwin(xps[1], 128, t),
                          lambda t: dwxw_t[:, 1, t:t + 1], xw[1][:, :, :])
                prod_tree(False, nc.gpsimd, "x2",
                          lambda t: xwin(xps[2], 64, t),
                          lambda t: dwxw_t[:64, 2, t:t + 1], xw[2][:64, :, :],
                          pc=64)
                return xw

            def emit_wy(n, h_sb, xw_sb):
                """i = Wy @ [h; xw] + bias -> i_pad interior rows (bf16)."""
                r0 = 8 * n
                for m in range(4):
                    ps = psi_p.tile([128, 512], F32, tag="psi", name="psi")
                    for k in range(4):
                        nc.tensor.matmul(
                            ps[:, :], wy_t[:, k, 128 * m:128 * (m + 1)],
                            h_sb[k][:, :], start=(k == 0), stop=False)
                    for j in range(3):
                        pc = XCH[j]
                        nc.tensor.matmul(
                            ps[:, :], wy_t[:pc, 4 + j, 128 * m:128 * (m + 1)],
                            xw_sb[j][:pc, :, :], start=False, stop=(j == 2))
                    nc.scalar.activation(
                        i_pad[m][:, 1 + r0:9 + r0, 1:65], ps[:, :],
                        AF.Identity, bias=wyb_t[:, m:m + 1], scale=1.0)

            # ---------------- prologue ----------------
            nc.sync.dma_start(out=dwxw_t[:], in_=dwxwd.ap())
            xp0 = dma_xpad(0)
            h0 = dma_h(0)
            nc.sync.dma_start(out=wy_t[:], in_=wyd.ap())
            nc.sync.dma_start(out=wyb_t[:], in_=wybd.ap())
            nc.sync.dma_start(out=dwiw_t[:], in_=dwiwd.ap())
            xp1 = dma_xpad(1)
            nc.sync.dma_start(out=wg_t[:], in_=wgd.ap())
            nc.sync.dma_start(out=dwid_t[:], in_=dwidd.ap())

            # zero i_pad halo borders (rows 0/65, cols 0/65)
            for m in range(4):
                eng = nc.vector if m < 2 else nc.gpsimd
                eng.memset(i_pad[m][:, 0, :], 0.0)
                eng.memset(i_pad[m][:, 65, :], 0.0)
                eng.memset(i_pad[m][:, 1:65, 0], 0.0)
                eng.memset(i_pad[m][:, 1:65, 65], 0.0)

            xw_sb = {0: emit_dwx(0, xp0)}
            h_sb = {0: h0}
            xp_sb = {1: xp1}
            bpend = {}  # slab -> acc pair for the DVE dwi chunk 0
            b_sb = {}   # slab -> [b0..b3]

            # ---------------- rounds ----------------
            for r in range(NCHUNK + 1):
                ra = r            # A-stage slab
                rb = r - 1        # B-stage slab
                rx = r + 1        # x-conv stage slab

                # DMAs for this round
                if 0 <= rb:
                    c_t = []
                    for m in range(4):
                        t = sB.tile([128, 512], BF, tag=f"c{m}", name=f"c{m}")
                        nc.sync.dma_start(
                            out=t[:],
                            in_=c_ap[128 * m:128 * (m + 1),
                                     512 * rb:512 * (rb + 1)])
                        c_t.append(t)
                if ra + 1 < NCHUNK:
                    h_sb[ra + 1] = dma_h(ra + 1)
                if rx + 1 < NCHUNK:
                    xp_sb[rx + 1] = dma_xpad(rx + 1)

                # PE: Wy matmuls + i_pad for slab ra. Must be EMITTED before
                # anything that reads slab ra's i_pad rows (Tile deps follow
                # emission order).
                if ra < NCHUNK:
                    emit_wy(ra, h_sb.pop(ra), xw_sb.pop(ra))

                # DVE: finish dwi chunk 0 of slab rb (taps 6..8; tap row
                # 8rb+9 is written by A(ra), whose m=0 i_pad lands ~2us in)
                if rb >= 0:
                    bt = [sB.tile([128, 8, 64], BF, tag=f"b{k}", name=f"b{k}")
                          for k in range(4)]
                    b_sb[rb] = bt
                    a0 = bpend.pop(rb)
                    stt_seg(a0, bt[0][:, :, :],
                            lambda t: iwin(0, rb, t),
                            lambda t: dwiw_t[:, 0, t:t + 1], 6, 9)

                # x-conv for slab rx (ACT products + Pool adds / DVE / DVE+Pool)
                # emitted before the diag copies so the ACT stream runs
                # [ipad, products, copies, gate-acts]
                if rx < NCHUNK:
                    xw_sb[rx] = emit_dwx(rx, xp_sb.pop(rx))

                # PE: dwi chunks 1..3 of slab rb as diag matmuls; ACT copies
                # PSUM -> SBUF bf16
                if rb >= 0:
                    for ci in range(1, 4):
                        psb = psb_p.tile([128, 8, 64], F32, tag="psb",
                                         name="psb")
                        for t in range(9):
                            nc.tensor.matmul(
                                psb[:, :, :],
                                dwid_t[:, ci - 1, 128 * t:128 * (t + 1)],
                                iwin(ci, rb, t), start=(t == 0), stop=(t == 8))
                        nc.scalar.copy(bt[ci][:, :, :], psb[:, :, :])

                # DVE: start dwi chunk 0 of slab ra (taps 0..5)
                if ra < NCHUNK:
                    a0 = new_accs("bacc0")
                    stt_seg(a0, None, lambda t: iwin(0, ra, t),
                            lambda t: dwiw_t[:, 0, t:t + 1], 0, 6)
                    bpend[ra] = a0

                # PE gates + ACT activations + DVE pointwise for slab rb.
                # LSTM trails gates by one m so the ACT stream never blocks
                # on a not-yet-computed cc; ch trails by two.
                if rb >= 0:
                    bt = b_sb.pop(rb)
                    sig = {}
                    u = {}
                    for m in range(6):
                        if m < 4:
                            sg = []
                            for g in range(4):
                                ps = psg_p.tile([128, 512], F32, tag="psg",
                                                name="psg")
                                for k in range(4):
                                    nc.tensor.matmul(
                                        ps[:, :],
                                        wg_t[:, 4 * g + k,
                                             128 * m:128 * (m + 1)],
                                        bt[k][:, :, :],
                                        start=(k == 0), stop=(k == 3))
                                st = sB.tile([128, 512], BF, tag=f"sg{g}",
                                             name=f"sg{g}")
                                nc.scalar.activation(
                                    st[:, :], ps[:, :],
                                    AF.Relu if g == 2 else AF.Sigmoid)
                                sg.append(st)
                            sig[m] = sg
                        if 1 <= m <= 4:
                            mm = m - 1
                            sg = sig.pop(mm)
                            u1 = sB.tile([128, 512], BF, tag="u1", name="u1")
                            nc.vector.tensor_mul(u1[:, :], sg[1][:, :],
                                                 c_t[mm][:, :])
                            u2 = sB.tile([128, 512], BF, tag="u2", name="u2")
                            nc.vector.scalar_tensor_tensor(
                                out=u2[:, :], in0=sg[2][:, :], scalar=6.0,
                                in1=sg[0][:, :], op0=ALU.min, op1=ALU.mult)
                            cc_t = sB.tile([128, 512], F32, tag="cc",
                                           name="cc")
                            nc.vector.tensor_add(cc_t[:, :], u1[:, :],
                                                 u2[:, :])
                            nc.sync.dma_start(
                                out=cc_ap[128 * mm:128 * (mm + 1),
                                          512 * rb:512 * (rb + 1)],
                                in_=cc_t[:])
                            rcc = sB.tile([128, 512], BF, tag="rcc",
                                          name="rcc")
                            nc.scalar.activation(rcc[:, :], cc_t[:, :],
                                                 AF.Relu)
                            u[mm] = (rcc, sg[3])
                        if 2 <= m <= 5:
                            mm = m - 2
                            rcc, sg3 = u.pop(mm)
                            ch_t = sB.tile([128, 512], F32, tag="ch",
                                           name="ch")
                            nc.vector.scalar_tensor_tensor(
                                out=ch_t[:, :], in0=rcc[:, :], scalar=6.0,
                                in1=sg3[:, :], op0=ALU.min, op1=ALU.mult)
                            nc.sync.dma_start(
                                out=ch_ap[128 * mm:128 * (mm + 1),
                                          512 * rb:512 * (rb + 1)],
                                in_=ch_t[:])

    nc.compile()
    return nc


def pack_weights(W_dw, W_dwb, Wy, Wy_b, Wi, Wbi, Wbf, Wbc, Wbo):
    WyT = Wy[:, :, 0, 0].T.astype(np.float32)  # (832, 512) lhsT
    wy = np.zeros((128, 7, 512), np.float32)
    for k in range(4):  # h chunks first
        wy[:, k, :] = WyT[320 + 128 * k:320 + 128 * (k + 1), :]
    for k in range(2):
        wy[:, 4 + k, :] = WyT[128 * k:128 * (k + 1), :]
    wy[:64, 6, :] = WyT[256:320, :]

    wyb = (Wy_b + Wy[:, :320, 0, 0] @ W_dwb).astype(np.float32)
    wyb = np.ascontiguousarray(wyb.reshape(4, 128).T)

    wg = np.zeros((128, 16, 512), np.float32)
    for g, W in enumerate([Wbi, Wbf, Wbc, Wbo]):
        lhsT = W[:, :, 0, 0].T.astype(np.float32)  # (512 in, 512 out)
        for k in range(4):
            wg[:, 4 * g + k, :] = lhsT[128 * k:128 * (k + 1), :]

    wtap_x = W_dw[:, 0].reshape(CIN, 9)  # (c, t) tap-major (dy,dx)
    dwxw = np.zeros((128, 3, 9), np.float32)
    for ci in range(3):
        pc = XCH[ci]
        dwxw[:pc, ci, :] = wtap_x[128 * ci:128 * ci + pc, :]

    wtap_i = Wi[:, 0].reshape(CH, 9)
    dwiw = np.zeros((128, 4, 9), np.float32)
    for ci in range(4):
        dwiw[:, ci, :] = wtap_i[128 * ci:128 * (ci + 1), :]

    dwid = np.zeros((128, 3, 1152), np.float32)
    idx = np.arange(128)
    for ci in range(1, 4):
        for t in range(9):
            dwid[idx, ci - 1, 128 * t + idx] = wtap_i[128 * ci + idx, t]

    return {
        "wy": wy.astype(BF_NP), "wyb": wyb, "wg": wg.astype(BF_NP),
        "dwxw": dwxw, "dwiw": dwiw, "dwid": dwid.astype(BF_NP),
    }


_CACHE = {}


def _get_nc():
    if "nc" not in _CACHE:
        _CACHE["nc"] = build_nc()
    return _CACHE["nc"]


def run(inputs, trace=False, tmpdir=None):
    """inputs: dict as from setup_inputs(). Returns ((ch, cc), results_obj)."""
    inp = {k: np.asarray(v, np.float32) for k, v in inputs.items()}
    packed = pack_weights(
        inp["W_dw"], inp["W_dwb"], inp["Wy"], inp["Wy_b"], inp["Wi"],
        inp["Wbi"], inp["Wbf"], inp["Wbc"], inp["Wbo"],
    )
    xpad_host = np.zeros((B, CIN, 66, 66), np.float32)
    xpad_host[:, :, 1:65, 1:65] = inp["x"]
    xpad_host = xpad_host.astype(BF_NP)
    h_host = inp["h"].reshape(B, CH, PIX).astype(BF_NP)
    c_host = inp["c"].reshape(B, CH, PIX).astype(BF_NP)
    in_maps = []
    for b in range(B):
        in_maps.append({
            "x": xpad_host[b],
            "h": np.ascontiguousarray(h_host[b]),
            "c": np.ascontiguousarray(c_host[b]),
            **packed,
        })
    nc = _get_nc()
    kwargs = {}
    if trace:
        _enable_trace_hooks()
        kwargs = dict(trace=True, trace_cores=[0])
        if tmpdir:
            kwargs["tmpdir"] = tmpdir
    res = run_bass_kernel_spmd(nc, in_maps, core_ids=list(range(NCORES)), **kwargs)
    ch = np.stack([res.results[b]["och"].reshape(CH, HW, HW) for b in range(B)])
    cc = np.stack([res.results[b]["occ"].reshape(CH, HW, HW) for b in range(B)])
    return (ch, cc), res


def kernel(**inputs):
    (ch, cc), _ = run(inputs, trace=False)
    return ch, cc


# ---------- optional NTFF tracing support (test harness only) ----------

def _enable_trace_hooks():
    import types, ctypes, contextlib
    if "antenv.axon_hooks" in sys.modules:
        return
    import concourse.bass_utils as bass_utils

    def _ntff_profile_via_ctypes(so_path):
        lib = ctypes.CDLL(so_path)
        if not hasattr(lib, "axon_start_nrt_profile"):
            return None
        lib.axon_start_nrt_profile.argtypes = [
            ctypes.POINTER(ctypes.c_int64), ctypes.c_size_t]
        lib.axon_start_nrt_profile.restype = ctypes.c_int64
        lib.axon_stop_nrt_profile.argtypes = [ctypes.c_char_p]
        lib.axon_stop_nrt_profile.restype = ctypes.c_int64

        @contextlib.contextmanager
        def _hook(output_dir, device_ids):
            import jax
            jax.devices()
            if device_ids:
                ids = (ctypes.c_int64 * len(device_ids))(*device_ids)
                rc = lib.axon_start_nrt_profile(ids, len(device_ids))
            else:
                rc = lib.axon_start_nrt_profile(None, 0)
            if rc != 0:
                raise RuntimeError(f"axon_start_nrt_profile rc={rc}")
            try:
                yield
            finally:
                lib.axon_stop_nrt_profile(str(output_dir).encode())
        return _hook

    hook = _ntff_profile_via_ctypes("/opt/axon/libaxon_pjrt.so")
    mod = types.ModuleType("antenv.axon_hooks")
    mod.get_axon_ntff_profile_hook = lambda: hook
    mod.set_axon_ntff_profile_hook = lambda h: None
    sys.modules["antenv.axon_hooks"] = mod
    bass_utils.upload_artifacts = lambda tmpdir: "local://" + str(tmpdir)


# revision 9
# speedup vs baseline: 1.0589x; 1.0589x over previous
"""BottleneckLSTMCell fused kernel for 8 Trainium2 NeuronCores.

Sharding: data-parallel over batch (B=8 -> 1 image per core). Each core runs
the full cell for its image.

v2.2 (engine budget per round, measured costs): PE keeps the dense matmuls
(Wy 28 + gates 64, bf16, 216ns each) plus all four dwi depthwise chunks as
diag matmuls (9 x 216ns per chunk -- much cheaper than DVE's ~850ns/tap
scalar_tensor_tensor). The dwx conv comes off PE: chunk 0 as ACT products
(activation Copy with per-channel scale) + gpsimd add chain, chunks 1/2 as
DVE scalar_tensor_tensor chains (tap 0 folded via a zeros tile -- DVE
tensor_scalar on strided windows measured slow, so it is avoided). LSTM
pointwise: u1 on gpsimd, u2/cc/ch on DVE, sigmoids/relu/rcc + the four
PSUM->SBUF b copies on ACT. Rounds pipeline phase A of slab r with phase B
of slab r-1 and the x-conv of slab r+1.
"""

import sys

if '/opt/trn_rl_repo' not in sys.path:
    sys.path.insert(0, '/opt/trn_rl_repo')

import numpy as np
import ml_dtypes

import concourse.bass as bass  # noqa: F401
from concourse import bacc
import concourse.mybir as mybir
from concourse.tile import TileContext
from concourse.bass_utils import run_bass_kernel_spmd

F32 = mybir.dt.float32
BF = mybir.dt.bfloat16
AF = mybir.ActivationFunctionType
ALU = mybir.AluOpType
BF_NP = ml_dtypes.bfloat16

B, CIN, CH, HW = 8, 320, 512, 64
PIX = HW * HW          # 4096
NCORES = 8
NCHUNK = 8             # spatial slabs of 8 rows (512 px)
XCH = [128, 128, 64]   # x channel chunk sizes (320)

TAPS = [(t // 3 - 1, t % 3 - 1) for t in range(9)]


def build_nc():
    nc = bacc.Bacc(None, target_bir_lowering=False, num_devices=NCORES)

    xd = nc.dram_tensor("x", (CIN, 66, 66), BF, kind="ExternalInput")
    hd = nc.dram_tensor("h", (CH, PIX), BF, kind="ExternalInput")
    cd = nc.dram_tensor("c", (CH, PIX), BF, kind="ExternalInput")
    wyd = nc.dram_tensor("wy", (128, 7, 512), BF, kind="ExternalInput")
    wybd = nc.dram_tensor("wyb", (128, 4), F32, kind="ExternalInput")
    wgd = nc.dram_tensor("wg", (128, 16, 512), BF, kind="ExternalInput")
    dwxwd = nc.dram_tensor("dwxw", (128, 3, 9), F32, kind="ExternalInput")
    dwidd = nc.dram_tensor("dwid", (128, 4, 1152), BF, kind="ExternalInput")
    ccd = nc.dram_tensor("occ", (CH, PIX), F32, kind="ExternalOutput")
    chd = nc.dram_tensor("och", (CH, PIX), F32, kind="ExternalOutput")

    x_ap, h_ap, c_ap = xd.ap(), hd.ap(), cd.ap()
    cc_ap, ch_ap = ccd.ap(), chd.ap()

    with TileContext(nc) as tc:
        with tc.tile_pool(name="persist", bufs=1) as pp, \
             tc.tile_pool(name="sA", bufs=2) as sA, \
             tc.tile_pool(name="sB", bufs=2) as sB, \
             tc.tile_pool(name="psi", bufs=2, space="PSUM") as psi_p, \
             tc.tile_pool(name="psg", bufs=4, space="PSUM") as psg_p, \
             tc.tile_pool(name="psb", bufs=2, space="PSUM") as psb_p:

            # ------------- persistent tiles -------------
            i_pad = [pp.tile([128, 66, 66], BF, tag=f"ipad{m}", name=f"ipad{m}")
                     for m in range(4)]
            wy_t = pp.tile([128, 7, 512], BF, tag="wy", name="wy")
            wyb_t = pp.tile([128, 4], F32, tag="wyb", name="wyb")
            wg_t = pp.tile([128, 16, 512], BF, tag="wg", name="wg")
            dwxw_t = pp.tile([128, 3, 9], F32, tag="dwxw", name="dwxw")
            dwid_t = pp.tile([128, 4, 1152], BF, tag="dwid", name="dwid")
            zeros_t = pp.tile([128, 8, 64], F32, tag="zeros", name="zeros")

            # ------------- helpers -------------
            def dma_h(n):
                ts = []
                for k in range(4):
                    t = sA.tile([128, 512], BF, tag=f"h{k}", name=f"h{k}")
                    nc.sync.dma_start(
                        out=t[:],
                        in_=h_ap[128 * k:128 * (k + 1), 512 * n:512 * (n + 1)])
                    ts.append(t)
                return ts

            def dma_xpad(n):
                ts = []
                for ci in range(3):
                    pc = XCH[ci]
                    xp = sA.tile([128, 10, 66], BF, tag=f"xp{ci}",
                                 name=f"xp{ci}")
                    nc.sync.dma_start(
                        out=xp[:pc, :, :],
                        in_=x_ap[128 * ci:128 * ci + pc, 8 * n:8 * n + 10, :])
                    ts.append(xp)
                return ts

            def xwin(xp, pc, t):
                dy, dx = TAPS[t]
                return xp[:pc, 1 + dy:9 + dy, 1 + dx:65 + dx]

            def iwin(ci, n, t):
                dy, dx = TAPS[t]
                r0 = 8 * n
                return i_pad[ci][:, 1 + r0 + dy:9 + r0 + dy, 1 + dx:65 + dx]

            def stt_chain(tag, out_bf, win, w_ap, pc=128):
                """9-tap chain on DVE. Tap 0 reads a zeros tile as in1 (the
                strided-window tensor_scalar path measured ~1.2us, so every
                tap is the same scalar_tensor_tensor form). fp32 ping-pong
                accumulators; tap 8 writes out_bf."""
                accs = [sA.tile([128, 8, 64], F32, tag=f"{tag}{j}",
                                name=f"{tag}{j}") for j in range(2)]
                for t in range(9):
                    src = zeros_t[:pc, :, :] if t == 0 else \
                        accs[(t + 1) % 2][:pc, :, :]
                    dst = out_bf if t == 8 else accs[t % 2][:pc, :, :]
                    nc.vector.scalar_tensor_tensor(
                        out=dst, in0=win(t), scalar=w_ap(t), in1=src,
                        op0=ALU.mult, op1=ALU.add)

            def emit_dwx(n, xps):
                """27 x-conv taps for slab n -> xw tiles (bf16).

                ci0: ACT products + gpsimd add chain; ci1/ci2: DVE chains."""
                xw = [sA.tile([128, 8, 64], BF, tag=f"xw{ci}", name=f"xw{ci}")
                      for ci in range(3)]
                prod = sA.tile([128, 9, 512], BF, tag="prod", name="prod",
                               bufs=1)
                for t in range(9):
                    nc.scalar.activation(
                        prod[:, t, :], xwin(xps[0], 128, t), AF.Copy,
                        scale=dwxw_t[:, 0, t:t + 1])
                tmps = [sA.tile([128, 512], BF, tag=f"pt{j}", name=f"pt{j}")
                        for j in range(2)]
                nc.gpsimd.tensor_add(tmps[0][:, :], prod[:, 0, :],
                                     prod[:, 1, :])
                for j in range(2, 8):
                    nc.gpsimd.tensor_add(tmps[(j + 1) % 2][:, :],
                                         tmps[j % 2][:, :], prod[:, j, :])
                nc.gpsimd.tensor_add(xw[0][:, :, :], tmps[0][:, :],
                                     prod[:, 8, :])
                stt_chain("xa1", xw[1][:, :, :],
                          lambda t: xwin(xps[1], 128, t),
                          lambda t: dwxw_t[:, 1, t:t + 1])
                stt_chain("xa2", xw[2][:64, :, :],
                          lambda t: xwin(xps[2], 64, t),
                          lambda t: dwxw_t[:64, 2, t:t + 1], pc=64)
                return xw

            def emit_wy(n, h_sb, xw_sb):
                """i = Wy @ [h; xw] + bias -> i_pad interior rows (bf16)."""
                r0 = 8 * n
                for m in range(4):
                    ps = psi_p.tile([128, 512], F32, tag="psi", name="psi")
                    for k in range(4):
                        nc.tensor.matmul(
                            ps[:, :], wy_t[:, k, 128 * m:128 * (m + 1)],
                            h_sb[k][:, :], start=(k == 0), stop=False)
                    for j in range(3):
                        pc = XCH[j]
                        nc.tensor.matmul(
                            ps[:, :], wy_t[:pc, 4 + j, 128 * m:128 * (m + 1)],
                            xw_sb[j][:pc, :, :], start=False, stop=(j == 2))
                    nc.scalar.activation(
                        i_pad[m][:, 1 + r0:9 + r0, 1:65], ps[:, :],
                        AF.Identity, bias=wyb_t[:, m:m + 1], scale=1.0)

            # ---------------- prologue ----------------
            nc.sync.dma_start(out=dwxw_t[:], in_=dwxwd.ap())
            xp0 = dma_xpad(0)
            h0 = dma_h(0)
            nc.sync.dma_start(out=wy_t[:], in_=wyd.ap())
            nc.sync.dma_start(out=wyb_t[:], in_=wybd.ap())
            xp1 = dma_xpad(1)
            nc.sync.dma_start(out=wg_t[:], in_=wgd.ap())
            nc.sync.dma_start(out=dwid_t[:], in_=dwidd.ap())

            nc.vector.memset(zeros_t[:, :, :], 0.0)
            # zero i_pad halo borders (rows 0/65, cols 0/65)
            for m in range(4):
                eng = nc.vector if m < 2 else nc.gpsimd
                eng.memset(i_pad[m][:, 0, :], 0.0)
                eng.memset(i_pad[m][:, 65, :], 0.0)
                eng.memset(i_pad[m][:, 1:65, 0], 0.0)
                eng.memset(i_pad[m][:, 1:65, 65], 0.0)

            xw_sb = {0: emit_dwx(0, xp0)}
            h_sb = {0: h0}
            xp_sb = {1: xp1}

            # ---------------- rounds ----------------
            for r in range(NCHUNK + 1):
                ra = r            # A-stage slab
                rb = r - 1        # B-stage slab
                rx = r + 1        # x-conv stage slab

                # DMAs for this round
                if rb >= 0:
                    c_t = []
                    for m in range(4):
                        t = sB.tile([128, 512], BF, tag=f"c{m}", name=f"c{m}")
                        nc.sync.dma_start(
                            out=t[:],
                            in_=c_ap[128 * m:128 * (m + 1),
                                     512 * rb:512 * (rb + 1)])
                        c_t.append(t)
                if ra + 1 < NCHUNK:
                    h_sb[ra + 1] = dma_h(ra + 1)
                if rx + 1 < NCHUNK:
                    xp_sb[rx + 1] = dma_xpad(rx + 1)

                # PE: Wy matmuls + i_pad for slab ra. Must be EMITTED before
                # anything reading slab ra's i_pad rows (Tile dependency
                # tracking follows emission order).
                if ra < NCHUNK:
                    emit_wy(ra, h_sb.pop(ra), xw_sb.pop(ra))

                # x-conv for slab rx: ACT products (first 6) before the b
                # copies so copies land just before the gate matmuls need
                # them; DVE chains + Pool adds via emit_dwx
                if rx < NCHUNK:
                    xw_sb[rx] = emit_dwx(rx, xp_sb.pop(rx))

                # PE: all four dwi chunks of slab rb as diag matmuls
                # (dy=+1 tap rows come from A(ra), emitted above); ACT
                # copies PSUM -> SBUF bf16
                if rb >= 0:
                    bt = []
                    for ci in range(4):
                        psb = psb_p.tile([128, 8, 64], F32, tag="psb",
                                         name="psb")
                        for t in range(9):
                            nc.tensor.matmul(
                                psb[:, :, :],
                                dwid_t[:, ci, 128 * t:128 * (t + 1)],
                                iwin(ci, rb, t), start=(t == 0), stop=(t == 8))
                        b = sB.tile([128, 8, 64], BF, tag=f"b{ci}",
                                    name=f"b{ci}")
                        nc.scalar.copy(b[:, :, :], psb[:, :, :])
                        bt.append(b)

                # PE gates + ACT activations + DVE/Pool pointwise for slab
                # rb. LSTM trails gates by one m so the ACT stream never
                # blocks on a not-yet-computed cc; ch trails by two.
                if rb >= 0:
                    sig = {}
                    u = {}
                    for m in range(6):
                        if m < 4:
                            sg = []
                            for g in range(4):
                                ps = psg_p.tile([128, 512], F32, tag="psg",
                                                name="psg")
                                for k in range(4):
                                    nc.tensor.matmul(
                                        ps[:, :],
                                        wg_t[:, 4 * g + k,
                                             128 * m:128 * (m + 1)],
                                        bt[k][:, :, :],
                                        start=(k == 0), stop=(k == 3))
                                st = sB.tile([128, 512], BF, tag=f"sg{g}",
                                             name=f"sg{g}")
                                nc.scalar.activation(
                                    st[:, :], ps[:, :],
                                    AF.Relu if g == 2 else AF.Sigmoid)
                                sg.append(st)
                            sig[m] = sg
                        if 1 <= m <= 4:
                            mm = m - 1
                            sg = sig.pop(mm)
                            u1 = sB.tile([128, 512], BF, tag="u1", name="u1")
                            nc.gpsimd.tensor_mul(u1[:, :], sg[1][:, :],
                                                 c_t[mm][:, :])
                            u2 = sB.tile([128, 512], BF, tag="u2", name="u2")
                            nc.vector.scalar_tensor_tensor(
                                out=u2[:, :], in0=sg[2][:, :], scalar=6.0,
                                in1=sg[0][:, :], op0=ALU.min, op1=ALU.mult)
                            cc_t = sB.tile([128, 512], F32, tag="cc",
                                           name="cc")
                            nc.vector.tensor_add(cc_t[:, :], u1[:, :],
                                                 u2[:, :])
                            nc.sync.dma_start(
                                out=cc_ap[128 * mm:128 * (mm + 1),
                                          512 * rb:512 * (rb + 1)],
                                in_=cc_t[:])
                            rcc = sB.tile([128, 512], BF, tag="rcc",
                                          name="rcc")
                            nc.scalar.activation(rcc[:, :], cc_t[:, :],
                                                 AF.Relu)
                            u[mm] = (rcc, sg[3])
                        if 2 <= m <= 5:
                            mm = m - 2
                            rcc, sg3 = u.pop(mm)
                            ch_t = sB.tile([128, 512], F32, tag="ch",
                                           name="ch")
                            nc.vector.scalar_tensor_tensor(
                                out=ch_t[:, :], in0=rcc[:, :], scalar=6.0,
                                in1=sg3[:, :], op0=ALU.min, op1=ALU.mult)
                            nc.sync.dma_start(
                                out=ch_ap[128 * mm:128 * (mm + 1),
                                          512 * rb:512 * (rb + 1)],
                                in_=ch_t[:])

    nc.compile()
    return nc


def pack_weights(W_dw, W_dwb, Wy, Wy_b, Wi, Wbi, Wbf, Wbc, Wbo):
    WyT = Wy[:, :, 0, 0].T.astype(np.float32)  # (832, 512) lhsT
    wy = np.zeros((128, 7, 512), np.float32)
    for k in range(4):  # h chunks first
        wy[:, k, :] = WyT[320 + 128 * k:320 + 128 * (k + 1), :]
    for k in range(2):
        wy[:, 4 + k, :] = WyT[128 * k:128 * (k + 1), :]
    wy[:64, 6, :] = WyT[256:320, :]

    wyb = (Wy_b + Wy[:, :320, 0, 0] @ W_dwb).astype(np.float32)
    wyb = np.ascontiguousarray(wyb.reshape(4, 128).T)

    wg = np.zeros((128, 16, 512), np.float32)
    for g, W in enumerate([Wbi, Wbf, Wbc, Wbo]):
        lhsT = W[:, :, 0, 0].T.astype(np.float32)  # (512 in, 512 out)
        for k in range(4):
            wg[:, 4 * g + k, :] = lhsT[128 * k:128 * (k + 1), :]

    wtap_x = W_dw[:, 0].reshape(CIN, 9)  # (c, t) tap-major (dy,dx)
    dwxw = np.zeros((128, 3, 9), np.float32)
    for ci in range(3):
        pc = XCH[ci]
        dwxw[:pc, ci, :] = wtap_x[128 * ci:128 * ci + pc, :]

    wtap_i = Wi[:, 0].reshape(CH, 9)
    dwid = np.zeros((128, 4, 1152), np.float32)
    idx = np.arange(128)
    for ci in range(4):
        for t in range(9):
            dwid[idx, ci, 128 * t + idx] = wtap_i[128 * ci + idx, t]

    return {
        "wy": wy.astype(BF_NP), "wyb": wyb, "wg": wg.astype(BF_NP),
        "dwxw": dwxw, "dwid": dwid.astype(BF_NP),
    }


_CACHE = {}


def _get_nc():
    if "nc" not in _CACHE:
        _CACHE["nc"] = build_nc()
    return _CACHE["nc"]


def run(inputs, trace=False, tmpdir=None):
    """inputs: dict as from setup_inputs(). Returns ((ch, cc), results_obj)."""
    inp = {k: np.asarray(v, np.float32) for k, v in inputs.items()}
    packed = pack_weights(
        inp["W_dw"], inp["W_dwb"], inp["Wy"], inp["Wy_b"], inp["Wi"],
        inp["Wbi"], inp["Wbf"], inp["Wbc"], inp["Wbo"],
    )
    xpad_host = np.zeros((B, CIN, 66, 66), np.float32)
    xpad_host[:, :, 1:65, 1:65] = inp["x"]
    xpad_host = xpad_host.astype(BF_NP)
    h_host = inp["h"].reshape(B, CH, PIX).astype(BF_NP)
    c_host = inp["c"].reshape(B, CH, PIX).astype(BF_NP)
    in_maps = []
    for b in range(B):
        in_maps.append({
            "x": xpad_host[b],
            "h": np.ascontiguousarray(h_host[b]),
            "c": np.ascontiguousarray(c_host[b]),
            **packed,
        })
    nc = _get_nc()
    kwargs = {}
    if trace:
        _enable_trace_hooks()
        kwargs = dict(trace=True, trace_cores=[0])
        if tmpdir:
            kwargs["tmpdir"] = tmpdir
    res = run_bass_kernel_spmd(nc, in_maps, core_ids=list(range(NCORES)), **kwargs)
    ch = np.stack([res.results[b]["och"].reshape(CH, HW, HW) for b in range(B)])
    cc = np.stack([res.results[b]["occ"].reshape(CH, HW, HW) for b in range(B)])
    return (ch, cc), res


def kernel(**inputs):
    (ch, cc), _ = run(inputs, trace=False)
    return ch, cc


# ---------- optional NTFF tracing support (test harness only) ----------

def _enable_trace_hooks():
    import types, ctypes, contextlib
    if "antenv.axon_hooks" in sys.modules:
        return
    import concourse.bass_utils as bass_utils

    def _ntff_profile_via_ctypes(so_path):
        lib = ctypes.CDLL(so_path)
        if not hasattr(lib, "axon_start_nrt_profile"):
            return None
        lib.axon_start_nrt_profile.argtypes = [
            ctypes.POINTER(ctypes.c_int64), ctypes.c_size_t]
        lib.axon_start_nrt_profile.restype = ctypes.c_int64
        lib.axon_stop_nrt_profile.argtypes = [ctypes.c_char_p]
        lib.axon_stop_nrt_profile.restype = ctypes.c_int64

        @contextlib.contextmanager
        def _hook(output_dir, device_ids):
            import jax
            jax.devices()
            if device_ids:
                ids = (ctypes.c_int64 * len(device_ids))(*device_ids)
                rc = lib.axon_start_nrt_profile(ids, len(device_ids))
            else:
                rc = lib.axon_start_nrt_profile(None, 0)
            if rc != 0:
                raise RuntimeError(f"axon_start_nrt_profile rc={rc}")
            try:
                yield
            finally:
                lib.axon_stop_nrt_profile(str(output_dir).encode())
        return _hook

    hook = _ntff_profile_via_ctypes("/opt/axon/libaxon_pjrt.so")
    mod = types.ModuleType("antenv.axon_hooks")
    mod.get_axon_ntff_profile_hook = lambda: hook
    mod.set_axon_ntff_profile_hook = lambda h: None
    sys.modules["antenv.axon_hooks"] = mod
    bass_utils.upload_artifacts = lambda tmpdir: "local://" + str(tmpdir)
